# revision 57
# baseline (speedup 1.0000x reference)
"""EntityExtractor Trainium2 kernel.

Full-input contract: kernel(**inputs) takes the unsharded inputs of
reference.setup_inputs() and returns the same 9-tuple as reference.reference().
Internally shards batch rows across 8 NeuronCores (data parallel, one row per
core), runs one Bass/Tile NEFF on all cores SPMD, and reassembles on host.
"""
import sys

sys.path.insert(0, "/opt/trn_rl_repo")

import numpy as np
import concourse.bass as bass
import concourse.mybir as mybir
from concourse.bass_utils import run_bass_kernel_spmd
from concourse.tile import TileContext

F32 = mybir.dt.float32
U32 = mybir.dt.uint32
AF = mybir.ActivationFunctionType
OP = mybir.AluOpType

B, S, D, H = 8, 4096, 1024, 1024
K = 20
T = 10
MIN_SCORE = 0.4
EPS = 1e-5
NT = S // 128          # 32 token tiles of 128
NC_ = D // 128         # 8 contraction chunks
N_CORES = 8


def _split_multi_waits(nc):
    """This walrus build accepts at most one sync-wait per instruction; hoist
    extras onto injected same-engine NOPs placed immediately before."""
    for f in nc.m.functions:
        for bb in f.blocks:
            new_insts = []
            for ins in bb.instructions:
                si = ins.sync_info
                if si is not None and si.on_wait and len(si.on_wait) > 1:
                    waits = list(si.on_wait)
                    for w in waits[:-1]:
                        nop = mybir.InstNoOp(
                            name=nc.get_next_instruction_name(),
                            ins=[], outs=[], engine=ins.engine)
                        nop.sync_info = mybir.SyncInfo(on_wait=[w], on_update=[])
                        nc.register_instruction(nop)
                        new_insts.append(nop)
                    si.on_wait = waits[-1:]
                new_insts.append(ins)
            bb.instructions = new_insts


def _build(nc, consts, trivial_affine, skip_sc_b1=False, skip_mlp_bias=False):
    import os
    PHASES = int(os.environ.get("KERNEL_PHASES", "4"))
    """Emit the per-core program. consts: dict of python floats (b2 scalars)."""
    dram_in = {}
    BF16_ = mybir.dt.bfloat16
    for entry in (
        ("x", [S, D]), ("xTh", [D, S], BF16_), ("xTl", [D, S], BF16_), ("mask", [1, S]),
        ("sc_b1", [1, H]), ("sc_w2rep", [128, H]),
        ("sc_w1h", [D, H], BF16_), ("sc_w1l", [D, H], BF16_),
        ("en_w1", [D, H]), ("en_b1", [1, H]), ("en_w2", [H, D]), ("en_b2", [1, D]),
        ("ty_w1", [D, H]), ("ty_b1", [1, H]), ("ty_w2", [H, T]), ("ty_b2", [1, T]),
        ("ident", [128, 128]), ("ones", [1, 128]), ("tokidx", [128, NT]),
    ):
        name, shape = entry[0], entry[1]
        dt_ = entry[2] if len(entry) > 2 else F32
        dram_in[name] = nc.dram_tensor(name, shape, dt_, kind="ExternalInput").ap()
    if not trivial_affine:
        for name in ("sc_g", "sc_be", "en_g", "en_be", "ty_g", "ty_be"):
            dram_in[name] = nc.dram_tensor(name, [128, H], F32, kind="ExternalInput").ap()

    dram_out = {}
    for name, shape in (
        ("tok_scores", [1, S]), ("topv", [1, K]), ("st", [1, K]), ("enp", [1, K]),
        ("vk", [1, K]), ("enhanced", [K, D]), ("logits", [K, T]),
        ("probs", [K, T]), ("types", [K, 8]),
    ):
        dt = U32 if name == "types" else F32
        dram_out[name] = nc.dram_tensor(name, shape, dt, kind="ExternalOutput").ap()

    sc_b2 = consts["sc_b2"]

    with TileContext(nc, pool_alloc_mode="queue") as tc:
        with (
            tc.tile_pool(name="persist", bufs=1) as pp,
            tc.tile_pool(name="ps_small", bufs=1, space="PSUM") as psmall,
        ):
            ident = pp.tile([128, 128], F32, tag="ident")
            nc.sync.dma_start(out=ident[:], in_=dram_in["ident"][:])
            ones = pp.tile([1, 128], F32, tag="ones")
            nc.sync.dma_start(out=ones[:], in_=dram_in["ones"][:])
            tokidx = pp.tile([128, NT], F32, tag="tokidx")
            nc.sync.dma_start(out=tokidx[:], in_=dram_in["tokidx"][:])
            w2rep = pp.tile([128, H], F32, tag="w2rep")
            nc.sync.dma_start(out=w2rep[:], in_=dram_in["sc_w2rep"][:])
            maskarr = pp.tile([128, NT], F32, tag="maskarr")
            nc.sync.dma_start(out=maskarr[:],
                              in_=dram_in["mask"][0, :].rearrange("(i p) -> p i", p=128))
            z = pp.tile([128, NT], F32, tag="z")
            # winner/span rows that must survive across phase pools
            reps = pp.tile([128, 4, K], F32, tag="reps")
            st_rep = reps[:, 0, :]
            en_rep = reps[:, 1, :]
            sc_rep = reps[:, 2, :]
            vk_rep = reps[:, 3, :]
            smT = pp.tile([128, NC_, K], F32, tag="smT")
            wins = pp.tile([1, 24], F32, tag="wins")
            vk = pp.tile([1, K], F32, tag="vk")
            strow = pp.tile([1, K], F32, tag="strow")
            enrow = pp.tile([1, K], F32, tag="enrow")
            scoresM = pp.tile([128, NT], F32, tag="scoresM")

            if not trivial_affine:
                affine = {}
                for name in ("sc_g", "sc_be", "en_g", "en_be", "ty_g", "ty_be"):
                    affine[name] = pp.tile([128, H], F32, tag=name, name=name + "_aff")
                    nc.sync.dma_start(out=affine[name][:], in_=dram_in[name][:])

            # x re-stream prefetch for the span phase (ring-early so mlpw fits;
            # 6 slots -> DMAs trickle in during phases 1-2, rest pipeline in phase 3)
            span_x_pool = tc.tile_pool(name="spansb", bufs=16)
            ssb = span_x_pool.__enter__()
            xt2_tiles = []
            for _i in range(NT):
                xt2 = ssb.tile([128, D], F32, tag="xt2")
                nc.gpsimd.dma_start(out=xt2[:], in_=dram_in["x"][_i * 128:(_i + 1) * 128, :])
                xt2_tiles.append(xt2)

            # ---------------- Phase 1: scorer MLP over 32 token tiles ----------
            with (
                tc.tile_pool(name="mainw", bufs=1) as mwp,
                tc.tile_pool(name="mainsb", bufs=3) as msb,
                tc.tile_pool(name="mainps", bufs=2, space="PSUM") as mps,
            ):
                BF16 = mybir.dt.bfloat16
                w1h = mwp.tile([128, NC_, H], BF16, tag="w1h")
                nc.sync.dma_start(
                    out=w1h[:], in_=dram_in["sc_w1h"].rearrange("(c p) h -> p c h", p=128))
                w1l = mwp.tile([128, NC_, H], BF16, tag="w1l")
                nc.sync.dma_start(
                    out=w1l[:], in_=dram_in["sc_w1l"].rearrange("(c p) h -> p c h", p=128))
                b1row = mwp.tile([1, H], F32, tag="b1row")
                nc.sync.dma_start(out=b1row[:], in_=dram_in["sc_b1"][:])

                for i in range(NT):
                    # host-pretransposed, host-presplit bf16 hi/lo x slices
                    xth = msb.tile([128, NC_, 128], BF16, tag="xth")
                    nc.sync.dma_start(
                        out=xth[:],
                        in_=dram_in["xTh"][:, i * 128:(i + 1) * 128].rearrange(
                            "(c p) t -> p c t", p=128))
                    xtl = msb.tile([128, NC_, 128], BF16, tag="xtl")
                    nc.sync.dma_start(
                        out=xtl[:],
                        in_=dram_in["xTl"][:, i * 128:(i + 1) * 128].rearrange(
                            "(c p) t -> p c t", p=128))
                    # h = x @ w1 + b1 via bf16 hi/lo 3-pass -> psum, one tile per half
                    hps0 = mps.tile([128, 512], F32, tag="hps0")
                    hps1 = mps.tile([128, 512], F32, tag="hps1")
                    for hf, hps_t in ((0, hps0), (1, hps1)):
                        mms = [(xa, wb, c) for (xa, wb) in ((xth, w1h), (xth, w1l), (xtl, w1h))
                               for c in range(NC_)]
                        for j, (xa, wb, c) in enumerate(mms):
                            nc.tensor.matmul(hps_t[:], xa[:, c, :],
                                             wb[:, c, hf * 512:(hf + 1) * 512],
                                             start=(j == 0),
                                             stop=(skip_sc_b1 and j == len(mms) - 1))
                        if not skip_sc_b1:
                            nc.tensor.matmul(hps_t[:], ones[0:1, :],
                                             b1row[0:1, hf * 512:(hf + 1) * 512],
                                             start=False, stop=True)
                    # LN stats
                    bns = msb.tile([128, 2, 6], F32, tag="bns")
                    nc.vector.bn_stats(bns[:, 0, :], hps0[:])
                    nc.vector.bn_stats(bns[:, 1, :], hps1[:])
                    mv = msb.tile([128, 2], F32, tag="mv")
                    nc.vector.bn_aggr(mv[:], bns[:])
                    ve = msb.tile([128, 1], F32, tag="ve")
                    nc.vector.tensor_scalar(out=ve[:], in0=mv[:, 1:2], scalar1=EPS,
                                            scalar2=None, op0=OP.add)
                    sd = msb.tile([128, 1], F32, tag="sd")
                    nc.scalar.activation(out=sd[:], in_=ve[:], func=AF.Sqrt)
                    rstd = msb.tile([128, 1], F32, tag="rstd")
                    nc.vector.reciprocal(out=rstd[:], in_=sd[:])
                    bact = msb.tile([128, 1], F32, tag="bact")
                    nc.vector.scalar_tensor_tensor(out=bact[:], in0=mv[:, 0:1], scalar=-1.0,
                                                   in1=rstd[:], op0=OP.mult, op1=OP.mult)
                    gel = msb.tile([128, H], F32, tag="gel")
                    if trivial_affine:
                        for hf, hps_t in ((0, hps0), (1, hps1)):
                            nc.scalar.activation(out=gel[:, hf * 512:(hf + 1) * 512],
                                                 in_=hps_t[:], func=AF.Gelu,
                                                 bias=bact[:], scale=rstd[:])
                    else:
                        hn = msb.tile([128, H], F32, tag="hn")
                        for hf, hps_t in ((0, hps0), (1, hps1)):
                            nc.scalar.activation(out=hn[:, hf * 512:(hf + 1) * 512],
                                                 in_=hps_t[:], func=AF.Copy,
                                                 bias=bact[:], scale=rstd[:])
                        nc.vector.tensor_tensor(out=hn[:], in0=hn[:], in1=affine["sc_g"][:], op=OP.mult)
                        nc.vector.tensor_tensor(out=hn[:], in0=hn[:], in1=affine["sc_be"][:], op=OP.add)
                        nc.scalar.activation(out=gel[:], in_=hn[:], func=AF.Gelu)
                    # dot with w2 -> z[:, i]
                    dump = msb.tile([128, H], F32, tag="dump")
                    nc.vector.scalar_tensor_tensor(out=dump[:], in0=gel[:], scalar=1.0,
                                                   in1=w2rep[:], op0=OP.bypass, op1=OP.mult,
                                                   accum_out=z[:, i:i + 1])

            if PHASES < 2:
                return
            # ---------------- Phase 2: scores, scans, topk --------------------
            with (
                tc.tile_pool(name="rowsb", bufs=1) as rsb,
                tc.tile_pool(name="rowps", bufs=1, space="PSUM") as rps,
            ):
                zb = rsb.tile([128, NT], F32, tag="zb")
                nc.vector.tensor_scalar(out=zb[:], in0=z[:], scalar1=sc_b2,
                                        scalar2=None, op0=OP.add)
                scores0 = rsb.tile([128, NT], F32, tag="scores0")
                nc.scalar.activation(out=scores0[:], in_=zb[:], func=AF.Sigmoid)
                nc.vector.tensor_tensor(out=scoresM[:], in0=scores0[:], in1=maskarr[:], op=OP.mult)
                act1 = rsb.tile([128, NT], F32, tag="act1")
                nc.vector.tensor_scalar(out=act1[:], in0=scoresM[:], scalar1=float(MIN_SCORE),
                                        scalar2=None, op0=OP.is_ge)
                act2 = rsb.tile([128, NT], F32, tag="act2")
                nc.vector.tensor_scalar(out=act2[:], in0=maskarr[:], scalar1=0.0,
                                        scalar2=None, op0=OP.is_gt)
                active = rsb.tile([128, NT], F32, tag="active")
                nc.vector.tensor_tensor(out=active[:], in0=act1[:], in1=act2[:], op=OP.mult)
                sa = rsb.tile([128, NT], F32, tag="sa")
                nc.vector.tensor_tensor(out=sa[:], in0=scoresM[:], in1=active[:], op=OP.mult)

                # transpose scoresM/active/sa to token-order rows
                pt = rps.tile([NT, 3, 128], F32, tag="pt")
                nc.tensor.transpose(pt[:, 0, :], scoresM[:], ident[:])
                nc.tensor.transpose(pt[:, 1, :], active[:], ident[:])
                nc.tensor.transpose(pt[:, 2, :], sa[:], ident[:])
                ptS = rsb.tile([NT, 3, 128], F32, tag="ptS")
                nc.vector.tensor_copy(ptS[:], pt[:])
                nc.sync.dma_start(out=dram_out["tok_scores"][:], in_=ptS[:, 0, :])
                # active on 8 lanes (p-major flatten of [32,128] = token order)
                C8 = S // 8
                act8 = rsb.tile([8, C8], F32, tag="act8")
                nc.sync.dma_start(out=act8[:], in_=ptS[:, 1, :])

                d0 = rsb.tile([2, S], F32, tag="d0")
                d1 = rsb.tile([2, S], F32, tag="d1")
                nc.sync.dma_start(out=d1[0:1, :], in_=ptS[:, 2, :])   # sa in token order
                nc.sync.dma_start(out=d1[1:2, :], in_=ptS[:, 1, :])   # active in token order
                # keep[t] = active[t]*active[t-1] built on 8 lanes; row-boundary fixup
                prevc = rsb.tile([8, 1], F32, tag="prevc")
                nc.vector.memset(prevc[:], 0.0)
                nc.sync.dma_start(out=prevc[1:8, :], in_=act8[0:7, C8 - 1:C8])
                keep8 = rsb.tile([8, C8], F32, tag="keep8")
                nc.vector.tensor_tensor(out=keep8[:, 1:], in0=act8[:, 1:],
                                        in1=act8[:, 0:C8 - 1], op=OP.mult)
                nc.vector.tensor_tensor(out=keep8[:, 0:1], in0=act8[:, 0:1],
                                        in1=prevc[:], op=OP.mult)
                nc.sync.dma_start(out=d0[0:1, :], in_=keep8[:])
                nc.sync.dma_start(out=d0[1:2, :], in_=keep8[:])
                scans = rsb.tile([2, S], F32, tag="scans")
                nc.vector.tensor_tensor_scan(out=scans[:], data0=d0[:], data1=d1[:],
                                             initial=0.0, op0=OP.mult, op1=OP.add)
                # end[t] = active[t] > active[t+1] on 8 lanes; row-boundary fixup
                nxtc = rsb.tile([8, 1], F32, tag="nxtc")
                nc.vector.memset(nxtc[:], 0.0)
                nc.sync.dma_start(out=nxtc[0:7, :], in_=act8[1:8, 0:1])
                end8 = rsb.tile([8, C8], F32, tag="end8")
                nc.vector.tensor_tensor(out=end8[:, 0:C8 - 1], in0=act8[:, 0:C8 - 1],
                                        in1=act8[:, 1:], op=OP.is_gt)
                nc.vector.tensor_tensor(out=end8[:, C8 - 1:], in0=act8[:, C8 - 1:],
                                        in1=nxtc[:], op=OP.is_gt)
                # R/L/end: [1,S] -> [32,128] -> PE transpose -> [128,32]
                R32 = rsb.tile([NT, 128], F32, tag="R32")
                nc.sync.dma_start(out=R32[:], in_=scans[0:1, :])
                L32 = rsb.tile([NT, 128], F32, tag="L32")
                nc.sync.dma_start(out=L32[:], in_=scans[1:2, :])
                E32 = rsb.tile([NT, 128], F32, tag="E32")
                nc.sync.dma_start(out=E32[:], in_=end8[:])
                ptA = rps.tile([128, 3, NT], F32, tag="ptA")
                nc.tensor.transpose(ptA[:, 0, :], R32[:], ident[0:NT, 0:NT])
                nc.tensor.transpose(ptA[:, 1, :], L32[:], ident[0:NT, 0:NT])
                nc.tensor.transpose(ptA[:, 2, :], E32[:], ident[0:NT, 0:NT])
                R128 = rsb.tile([128, NT], F32, tag="R128")
                nc.vector.tensor_copy(R128[:], ptA[:, 0, :])
                L128 = rsb.tile([128, NT], F32, tag="L128")
                nc.vector.tensor_copy(L128[:], ptA[:, 1, :])
                E128 = rsb.tile([128, NT], F32, tag="E128")
                nc.vector.tensor_copy(E128[:], ptA[:, 2, :])
                Lc128 = rsb.tile([128, NT], F32, tag="Lc128")
                nc.vector.tensor_scalar(out=Lc128[:], in0=L128[:], scalar1=1.0,
                                        scalar2=None, op0=OP.max)
                rL128 = rsb.tile([128, NT], F32, tag="rL128")
                nc.vector.reciprocal(out=rL128[:], in_=Lc128[:])
                avg128 = rsb.tile([128, NT], F32, tag="avg128")
                nc.vector.tensor_tensor(out=avg128[:], in0=R128[:], in1=rL128[:], op=OP.mult)
                A128 = rsb.tile([128, NT], F32, tag="A128")
                nc.vector.scalar_tensor_tensor(out=A128[:], in0=avg128[:], scalar=1.0,
                                               in1=E128[:], op0=OP.add, op1=OP.mult)
                nc.vector.tensor_scalar(out=A128[:], in0=A128[:], scalar1=-1.0,
                                        scalar2=None, op0=OP.add)
                # top-20 via 3 rounds of per-partition max8 + global max8
                A8 = rsb.tile([8, S // 8], F32, tag="A8")
                nc.sync.dma_start(out=A8[:], in_=A128[:])
                for r in range(3):
                    mx8 = rsb.tile([8, 8], F32, tag=f"mx8_{r}")
                    nc.vector.max(out=mx8[:], in_=A8[:])
                    cand = rsb.tile([1, 64], F32, tag=f"cand_{r}")
                    nc.sync.dma_start(out=cand[:], in_=mx8[:])
                    win8 = rsb.tile([1, 8], F32, tag=f"win8_{r}")
                    nc.vector.max(out=win8[:], in_=cand[:])
                    nc.vector.tensor_copy(wins[:, r * 8:(r + 1) * 8], win8[:])
                    if r < 2:
                        bps = rps.tile([8, 8], F32, tag=f"bps_{r}")
                        nc.tensor.matmul(bps[:], ones[0:1, 0:8], win8[:], start=True, stop=True)
                        bc8 = rsb.tile([8, 8], F32, tag=f"bc8_{r}")
                        nc.vector.tensor_copy(bc8[:], bps[:])
                        nc.vector.match_replace(out=A8[:], in_to_replace=bc8[:],
                                                in_values=A8[:], imm_value=-3.0)
                nc.vector.tensor_scalar(out=vk[:], in0=wins[:, 0:K], scalar1=float(MIN_SCORE),
                                        scalar2=None, op0=OP.is_ge)
                nc.sync.dma_start(out=dram_out["topv"][:], in_=wins[:, 0:K])
                nc.sync.dma_start(out=dram_out["vk"][:], in_=vk[:])


                # locate winners: e (end token) and cnt per winner
                wrps = rps.tile([128, K], F32, tag="wrps")
                nc.tensor.matmul(wrps[:], ones[0:1, :], wins[0:1, 0:K], start=True, stop=True)
                wrep = rsb.tile([128, K], F32, tag="wrep")
                nc.vector.tensor_copy(wrep[:], wrps[:])
                e_ps = rps.tile([1, K], F32, tag="e_ps")
                c_ps = rps.tile([1, K], F32, tag="c_ps")
                # batched match matrix M[p, i, k] = (A128[p,i] - w[k])^2 <= tol
                # via stride-0 broadcast APs, 3 DVE ops total
                MiA = rsb.tile([128, NT, K], F32, tag="MiA")
                a_b = A128[:].rearrange("p (i o) -> p i o", o=1).broadcast_to((128, NT, K))
                w_b = wrep[:].rearrange("p (o k) -> p o k", o=1).broadcast_to((128, NT, K))
                nc.vector.tensor_tensor(out=MiA[:], in0=a_b, in1=w_b, op=OP.subtract)
                nc.vector.tensor_tensor(out=MiA[:], in0=MiA[:], in1=MiA[:], op=OP.mult)
                nc.vector.tensor_scalar(out=MiA[:], in0=MiA[:], scalar1=2.5e-9,
                                        scalar2=None, op0=OP.is_le)
                for i in range(NT):
                    nc.tensor.matmul(e_ps[0:1, :], tokidx[:, i:i + 1], MiA[:, i, :],
                                     start=(i == 0), stop=(i == NT - 1))
                    nc.tensor.matmul(c_ps[0:1, :], L128[:, i:i + 1], MiA[:, i, :],
                                     start=(i == 0), stop=(i == NT - 1))
                erow = rsb.tile([1, K], F32, tag="erow")
                nc.vector.tensor_copy(erow[:], e_ps[0:1, :])
                cntrow = rsb.tile([1, K], F32, tag="cntrow")
                nc.vector.tensor_copy(cntrow[:], c_ps[0:1, :])
                # st = e - cnt + 1 ; masked by vk; cn = (cnt-1)*vk + 1
                st0 = rsb.tile([1, K], F32, tag="st0")
                nc.vector.scalar_tensor_tensor(out=st0[:], in0=cntrow[:], scalar=-1.0,
                                               in1=erow[:], op0=OP.mult, op1=OP.add)
                nc.vector.tensor_scalar(out=st0[:], in0=st0[:], scalar1=1.0,
                                        scalar2=None, op0=OP.add)
                nc.vector.tensor_tensor(out=strow[:], in0=st0[:], in1=vk[:], op=OP.mult)
                nc.vector.tensor_tensor(out=enrow[:], in0=erow[:], in1=vk[:], op=OP.mult)
                nc.sync.dma_start(out=dram_out["st"][:], in_=strow[:])
                nc.sync.dma_start(out=dram_out["enp"][:], in_=enrow[:])
                cn = rsb.tile([1, K], F32, tag="cn")
                nc.vector.tensor_scalar(out=cn[:], in0=cntrow[:], scalar1=-1.0,
                                        scalar2=None, op0=OP.add)
                nc.vector.tensor_tensor(out=cn[:], in0=cn[:], in1=vk[:], op=OP.mult)
                nc.vector.tensor_scalar(out=cn[:], in0=cn[:], scalar1=1.0,
                                        scalar2=None, op0=OP.add)
                rcn = rsb.tile([1, K], F32, tag="rcn")
                nc.vector.reciprocal(out=rcn[:], in_=cn[:])
                scrow = rsb.tile([1, K], F32, tag="scrow")
                nc.vector.tensor_tensor(out=scrow[:], in0=rcn[:], in1=vk[:], op=OP.mult)

                # broadcast st/en/scale/vk to [128, K] in ONE matmul
                rows4 = rsb.tile([1, 4, K], F32, tag="rows4")
                nc.vector.tensor_copy(rows4[:, 0, :], strow[:])
                nc.vector.tensor_copy(rows4[:, 1, :], enrow[:])
                nc.vector.tensor_copy(rows4[:, 2, :], scrow[:])
                nc.vector.tensor_copy(rows4[:, 3, :], vk[:])
                bbps = rps.tile([128, 4, K], F32, tag="bbps")
                nc.tensor.matmul(bbps[:].rearrange("p a b -> p (a b)"),
                                 ones[0:1, :], rows4[:].rearrange("p a b -> p (a b)"),
                                 start=True, stop=True)
                nc.vector.tensor_copy(reps[:], bbps[:])

            if PHASES < 3:
                return
            # MLP weights: load now so the DMAs overlap the span phase
            mlpw_pool = tc.tile_pool(name="mlpw", bufs=1)
            xwb = mlpw_pool.__enter__()
            ws = {}
            for wname, shape in (
                ("en_w1", [128, NC_, H]), ("en_w2", [128, NC_, D]),
                ("ty_w1", [128, NC_, H]), ("ty_w2", [128, NC_, T]),
            ):
                ws[wname] = xwb.tile(shape, F32, tag=wname, name=wname)
                nc.sync.dma_start(out=ws[wname][:],
                                  in_=dram_in[wname].rearrange("(c p) h -> p c h", p=128))
            rows = {}
            for rname, width in (("en_b1", H), ("en_b2", D), ("ty_b1", H), ("ty_b2", T)):
                rows[rname] = xwb.tile([1, width], F32, tag=rname, name=rname + "_r")
                nc.sync.dma_start(out=rows[rname][:], in_=dram_in[rname][:])

            # ---------------- Phase 3: span sums (prefetched x) ---------------
            with (
                tc.tile_pool(name="spanps", bufs=1, space="PSUM") as sps,
            ):
                sps_tiles = [sps.tile([128, K], F32, tag=f"sps_c{c}", name=f"sps_c{c}")
                             for c in range(NC_)]
                for i in range(NT):
                    xt2 = xt2_tiles[i]
                    i1 = ssb.tile([128, K], F32, tag="i1")
                    nc.vector.tensor_scalar(out=i1[:], in0=st_rep, scalar1=tokidx[:, i:i + 1],
                                            scalar2=None, op0=OP.is_le)
                    i2 = ssb.tile([128, K], F32, tag="i2")
                    nc.vector.tensor_scalar(out=i2[:], in0=en_rep, scalar1=tokidx[:, i:i + 1],
                                            scalar2=None, op0=OP.is_ge)
                    ind = ssb.tile([128, K], F32, tag="ind")
                    nc.vector.tensor_tensor(out=ind[:], in0=i1[:], in1=i2[:], op=OP.mult)
                    for c in range(NC_):
                        nc.tensor.matmul(sps_tiles[c][:], xt2[:, c * 128:(c + 1) * 128], ind[:],
                                         start=(i == 0), stop=(i == NT - 1))
                for c in range(NC_):
                    nc.vector.tensor_tensor(out=smT[:, c, :], in0=sps_tiles[c][:],
                                            in1=sc_rep, op=OP.mult)

            if PHASES < 4:
                return
            # ---------------- Phase 4: MLP_en, MLP_ty, softmax, argmax --------
            with (
                tc.tile_pool(name="mlpsb", bufs=1) as xsb,
                tc.tile_pool(name="mlpps", bufs=1, space="PSUM") as xps,
            ):

                def mlp_head(lhsT_chunks, w1, b1, g_name, be_name, nout):
                    """First linear + LN + gelu for a [K-span] head.
                    lhsT_chunks: [128, NC_, K] sbuf (input^T). Returns g sbuf [K, H].
                    Computed transposed (N=20 moving, 6x fewer PE cycles), then
                    PE-transposed back for the per-span LayerNorm."""
                    hT_ps = xps.tile([128, NC_, K], F32, tag="hT_ps")
                    for hc in range(NC_):
                        for kc in range(NC_):
                            nc.tensor.matmul(hT_ps[:, hc, :],
                                             w1[:, kc, hc * 128:(hc + 1) * 128],
                                             lhsT_chunks[:, kc, :],
                                             start=(kc == 0),
                                             stop=(skip_mlp_bias and kc == NC_ - 1))
                        if not skip_mlp_bias:
                            nc.tensor.matmul(hT_ps[:, hc, :],
                                             b1[0:1, hc * 128:(hc + 1) * 128],
                                             ones[0:1, 0:K], start=False, stop=True)
                    hT_s = xsb.tile([128, NC_, K], F32, tag="hT_s_" + g_name)
                    nc.vector.tensor_copy(hT_s[:], hT_ps[:])
                    hp = xps.tile([K, 2, 512], F32, tag="hp")
                    hp8 = hp[:].rearrange("p a (c b) -> p (a c) b", c=NC_ // 2)
                    for hc in range(NC_):
                        nc.tensor.transpose(hp8[:, hc, :], hT_s[:, hc, :], ident[:])
                    bns2 = xsb.tile([K, 2, 6], F32, tag="bns2_" + g_name)
                    nc.vector.bn_stats(bns2[:, 0, :], hp[:, 0, :])
                    nc.vector.bn_stats(bns2[:, 1, :], hp[:, 1, :])
                    mv2 = xsb.tile([K, 2], F32, tag="mv2_" + g_name)
                    nc.vector.bn_aggr(mv2[:], bns2[:])
                    ve2 = xsb.tile([K, 1], F32, tag="ve2_" + g_name)
                    nc.vector.tensor_scalar(out=ve2[:], in0=mv2[:, 1:2], scalar1=EPS,
                                            scalar2=None, op0=OP.add)
                    sd2 = xsb.tile([K, 1], F32, tag="sd2_" + g_name)
                    nc.scalar.activation(out=sd2[:], in_=ve2[:], func=AF.Sqrt)
                    rstd2 = xsb.tile([K, 1], F32, tag="rstd2_" + g_name)
                    nc.vector.reciprocal(out=rstd2[:], in_=sd2[:])
                    ba2 = xsb.tile([K, 1], F32, tag="ba2_" + g_name)
                    nc.vector.scalar_tensor_tensor(out=ba2[:], in0=mv2[:, 0:1], scalar=-1.0,
                                                   in1=rstd2[:], op0=OP.mult, op1=OP.mult)
                    g = xsb.tile([K, H], F32, tag="g_" + g_name)
                    if trivial_affine:
                        for hf in range(2):
                            nc.scalar.activation(out=g[:, hf * 512:(hf + 1) * 512],
                                                 in_=hp[:, hf, :], func=AF.Gelu,
                                                 bias=ba2[:], scale=rstd2[:])
                    else:
                        hn = xsb.tile([K, H], F32, tag="hn_" + g_name)
                        for hf in range(2):
                            nc.scalar.activation(out=hn[:, hf * 512:(hf + 1) * 512],
                                                 in_=hp[:, hf, :], func=AF.Copy,
                                                 bias=ba2[:], scale=rstd2[:])
                        nc.vector.tensor_tensor(out=hn[:], in0=hn[:],
                                                in1=affine[g_name][0:K, :], op=OP.mult)
                        nc.vector.tensor_tensor(out=hn[:], in0=hn[:],
                                                in1=affine[be_name][0:K, :], op=OP.add)
                        nc.scalar.activation(out=g[:], in_=hn[:], func=AF.Gelu)
                    return g

                # --- en head ---
                g1 = mlp_head(smT, ws["en_w1"], rows["en_b1"], "en_g", "en_be", D)
                g1T_ps = xps.tile([128, NC_, K], F32, tag="gT_ps")
                for c in range(NC_):
                    nc.tensor.transpose(g1T_ps[:, c, :], g1[:, c * 128:(c + 1) * 128], ident[0:K, 0:K])
                g1T = xsb.tile([128, NC_, K], F32, tag="g1T")
                nc.vector.tensor_copy(g1T[:], g1T_ps[:])
                enhT_ps = xps.tile([128, NC_, K], F32, tag="enhT_ps")
                for c in range(NC_):
                    for k in range(NC_):
                        nc.tensor.matmul(enhT_ps[:, c, :],
                                         ws["en_w2"][:, k, c * 128:(c + 1) * 128],
                                         g1T[:, k, :], start=(k == 0),
                                         stop=(skip_mlp_bias and k == NC_ - 1))
                    if not skip_mlp_bias:
                        nc.tensor.matmul(enhT_ps[:, c, :],
                                         rows["en_b2"][0:1, c * 128:(c + 1) * 128],
                                         ones[0:1, 0:K], start=False, stop=True)
                enhT = xsb.tile([128, NC_, K], F32, tag="enhT")
                for c in range(NC_):
                    nc.vector.tensor_tensor(out=enhT[:, c, :], in0=enhT_ps[:, c, :],
                                            in1=vk_rep, op=OP.mult)
                # enhanced natural layout for output
                enh_ps = xps.tile([K, NC_, 128], F32, tag="enh_ps")
                for c in range(NC_):
                    nc.tensor.transpose(enh_ps[:, c, :], enhT[:, c, :], ident[:])
                enh_s = xsb.tile([K, D], F32, tag="enh_s")
                nc.vector.tensor_copy(enh_s[:].rearrange("p (c f) -> p c f", c=NC_), enh_ps[:])
                nc.sync.dma_start(out=dram_out["enhanced"][:], in_=enh_s[:])

                # --- ty head ---
                g2 = mlp_head(enhT, ws["ty_w1"], rows["ty_b1"], "ty_g", "ty_be", T)
                g2T_ps = xps.tile([128, NC_, K], F32, tag="gT_ps")
                for c in range(NC_):
                    nc.tensor.transpose(g2T_ps[:, c, :], g2[:, c * 128:(c + 1) * 128], ident[0:K, 0:K])
                g2T = xsb.tile([128, NC_, K], F32, tag="g2T")
                nc.vector.tensor_copy(g2T[:], g2T_ps[:])
                lg_ps = xps.tile([K, T], F32, tag="lg_ps")
                for k in range(NC_):
                    nc.tensor.matmul(lg_ps[:], g2T[:, k, :], ws["ty_w2"][:, k, :],
                                     start=(k == 0),
                                     stop=(skip_mlp_bias and k == NC_ - 1))
                if not skip_mlp_bias:
                    nc.tensor.matmul(lg_ps[:], ones[0:1, 0:K], rows["ty_b2"][0:1, :],
                                     start=False, stop=True)
                lg = xsb.tile([K, T], F32, tag="lg")
                nc.vector.tensor_copy(lg[:], lg_ps[:])
                nc.sync.dma_start(out=dram_out["logits"][:], in_=lg[:])
                # softmax
                mxl = xsb.tile([K, 1], F32, tag="mxl")
                nc.vector.tensor_reduce(out=mxl[:], in_=lg[:], axis=mybir.AxisListType.X, op=OP.max)
                negm = xsb.tile([K, 1], F32, tag="negm")
                nc.vector.tensor_scalar(out=negm[:], in0=mxl[:], scalar1=-1.0,
                                        scalar2=None, op0=OP.mult)
                ex = xsb.tile([K, T], F32, tag="ex")
                se = xsb.tile([K, 1], F32, tag="se")
                nc.scalar.activation(out=ex[:], in_=lg[:], func=AF.Exp, bias=negm[:],
                                     scale=1.0, accum_out=se[:])
                rse = xsb.tile([K, 1], F32, tag="rse")
                nc.vector.reciprocal(out=rse[:], in_=se[:])
                pr = xsb.tile([K, T], F32, tag="pr")
                nc.vector.tensor_scalar(out=pr[:], in0=ex[:], scalar1=rse[:],
                                        scalar2=None, op0=OP.mult)
                nc.sync.dma_start(out=dram_out["probs"][:], in_=pr[:])
                # argmax via max8 + max_index
                mx8t = xsb.tile([K, 8], F32, tag="mx8t")
                nc.vector.max(out=mx8t[:], in_=lg[:])
                ix8 = xsb.tile([K, 8], U32, tag="ix8")
                nc.vector.max_index(out=ix8[:], in_max=mx8t[:], in_values=lg[:])
                nc.sync.dma_start(out=dram_out["types"][:], in_=ix8[:])
            mlpw_pool.__exit__(None, None, None)
            span_x_pool.__exit__(None, None, None)

    _split_multi_waits(nc)
    return nc


_CACHE = {}
last_results = None


def _get_program(trivial_affine, sc_b2_val, skip_sc_b1=False, skip_mlp_bias=False):
    key = (trivial_affine, float(sc_b2_val), skip_sc_b1, skip_mlp_bias)
    if key not in _CACHE:
        nc = bass.Bass("TRN2", target_bir_lowering=False, debug=False,
                       num_devices=N_CORES)
        _build(nc, {"sc_b2": float(sc_b2_val)}, trivial_affine, skip_sc_b1, skip_mlp_bias)
        _CACHE[key] = nc
    return _CACHE[key]


def kernel(hidden_states, attention_mask,
           sc_w1, sc_b1, sc_g, sc_be, sc_w2, sc_b2,
           en_w1, en_b1, en_g, en_be, en_w2, en_b2,
           ty_w1, ty_b1, ty_g, ty_be, ty_w2, ty_b2):
    hidden_states = np.ascontiguousarray(np.asarray(hidden_states, dtype=np.float32))
    attention_mask = np.ascontiguousarray(np.asarray(attention_mask, dtype=np.float32))
    f32 = lambda a: np.ascontiguousarray(np.asarray(a, dtype=np.float32))
    sc_w1, sc_b1, sc_g, sc_be, sc_w2 = map(f32, (sc_w1, sc_b1, sc_g, sc_be, sc_w2))
    en_w1, en_b1, en_g, en_be, en_w2, en_b2 = map(f32, (en_w1, en_b1, en_g, en_be, en_w2, en_b2))
    ty_w1, ty_b1, ty_g, ty_be, ty_w2, ty_b2 = map(f32, (ty_w1, ty_b1, ty_g, ty_be, ty_w2, ty_b2))
    sc_b2v = float(np.asarray(sc_b2).reshape(-1)[0])

    trivial = all(np.all(g == 1.0) for g in (sc_g, en_g, ty_g)) and \
        all(np.all(b == 0.0) for b in (sc_be, en_be, ty_be))

    skip_mlp_bias = bool(np.all(en_b1 == 0.0) and np.all(en_b2 == 0.0)
                         and np.all(ty_b1 == 0.0) and np.all(ty_b2 == 0.0))
    nc = _get_program(trivial, sc_b2v, skip_sc_b1=bool(np.all(sc_b1 == 0.0)),
                      skip_mlp_bias=skip_mlp_bias)

    ident = np.eye(128, dtype=np.float32)
    ones = np.ones((1, 128), dtype=np.float32)
    tokidx = (np.arange(128)[:, None] + 128 * np.arange(NT)[None, :]).astype(np.float32)
    w2rep = np.ascontiguousarray(np.broadcast_to(sc_w2.reshape(1, H), (128, H)))

    import ml_dtypes
    sc_w1h = sc_w1.astype(ml_dtypes.bfloat16)
    sc_w1l = (sc_w1 - sc_w1h.astype(np.float32)).astype(ml_dtypes.bfloat16)
    common = {
        "sc_w1h": sc_w1h, "sc_w1l": sc_w1l, "sc_b1": sc_b1.reshape(1, H), "sc_w2rep": w2rep,
        "en_w1": en_w1, "en_b1": en_b1.reshape(1, H), "en_w2": en_w2,
        "en_b2": en_b2.reshape(1, D),
        "ty_w1": ty_w1, "ty_b1": ty_b1.reshape(1, H), "ty_w2": ty_w2,
        "ty_b2": ty_b2.reshape(1, T),
        "ident": ident, "ones": ones, "tokidx": tokidx,
    }
    if not trivial:
        rep = lambda a: np.ascontiguousarray(np.broadcast_to(a.reshape(1, -1), (128, a.size)).astype(np.float32))
        common.update({"sc_g": rep(sc_g), "sc_be": rep(sc_be),
                       "en_g": rep(en_g), "en_be": rep(en_be),
                       "ty_g": rep(ty_g), "ty_be": rep(ty_be)})

    in_maps = []
    for b in range(N_CORES):
        m = dict(common)
        m["x"] = np.ascontiguousarray(hidden_states[b])
        xT = hidden_states[b].T
        xTh = xT.astype(ml_dtypes.bfloat16)
        m["xTh"] = np.ascontiguousarray(xTh)
        m["xTl"] = np.ascontiguousarray((xT - xTh.astype(np.float32)).astype(ml_dtypes.bfloat16))
        m["mask"] = np.ascontiguousarray(attention_mask[b].reshape(1, S))
        in_maps.append(m)

    import os
    trace = os.environ.get("KERNEL_TRACE") == "1"
    res = run_bass_kernel_spmd(nc, in_maps, list(range(N_CORES)), trace=trace)
    global last_results
    last_results = res

    enhanced = np.zeros((B, K, D), np.float32)
    logits = np.zeros((B, K, T), np.float32)
    probs = np.zeros((B, K, T), np.float32)
    types = np.zeros((B, K), np.int32)
    span_scores = np.zeros((B, K), np.float32)
    st = np.zeros((B, K), np.int32)
    enp = np.zeros((B, K), np.int32)
    valid_k = np.zeros((B, K), bool)
    tok_scores = np.zeros((B, S), np.float32)
    for b in range(N_CORES):
        r = res.results[b]
        enhanced[b] = r["enhanced"]
        logits[b] = r["logits"]
        probs[b] = r["probs"]
        types[b] = r["types"][:, 0].astype(np.int32)
        span_scores[b] = r["topv"][0]
        st[b] = np.rint(r["st"][0]).astype(np.int32)
        enp[b] = np.rint(r["enp"][0]).astype(np.int32)
        valid_k[b] = r["vk"][0] > 0.5
        tok_scores[b] = r["tok_scores"][0]
    return (enhanced, logits, probs, types, span_scores, st, enp, valid_k, tok_scores)
